# revision 21
# baseline (speedup 1.0000x reference)
"""Causal self-attention on 8 Trainium2 NeuronCores.

Problem: x[4, 2048, 1024] f32, W_attn[1024, 3072], b_attn[3072],
W_proj[1024, 1024], b_proj[1024];  16 heads, head_dim 64.

Sharding (data + tensor parallel, Megatron-style):
  core c = (b, g), b = c // 2 (batch), g = c % 2 (head group of 8 heads).
  - QKV weights column-sharded: core computes q,k,v for its 8 heads only.
  - W_proj row-sharded: core computes a partial [T, C] projection.
  - Host gathers: out[b] = partial[b,g=0] + partial[b,g=1] + b_proj.

Device layouts (per core):
  xTq  [128, 512] bf16 x (8 ct x 4 tq)  (x[b]^T quartered along T so the
                          first GEMM chunk only waits on 1/4 of the DMA)
  qkT  [1024, 2048] bf16 in SBUF: q rows 0-511, k rows 512-1023 (per-head
                          64-partition slabs -> ready as matmul operands)
  v    [2048, 1024] bf16: per head h a 128-col block [v_h (64) | ones (64)]
                          so the AV matmul lhsT yields PSUM rows 0-63 = y^T
                          and rows 64-127 = the softmax denominator
                          replicated 64x (free partition broadcast for the
                          divide).
  Causal: only blocks j <= i computed; diagonal 128x128 blocks masked by
  elementwise multiply with an upper-triangular 0/1 tile after exp.

Scheduling (v2): the ScalarE exp paces the attention inner loop (~0.85us
per j-tile vs ~0.6us of PE work), so PE micro-stalls every tile and each
stall drops the PE clock (p-state) for ~3us.  Fix: all GEMM PSUM chunks
are [128, 512] double-buffered (no DVE-add convoys), and the deferred
QKV GEMMs are paced INTO the attention stream one chunk per ~5 j-tiles
(with demand-driven flushes before each head-pair) so the PE queue always
holds runnable work.  The softmax normalization uses
reciprocal_approx_fast (1.3us vs 6.5us for the exact reciprocal, which
burned 105us of DVE and risked pacing the whole pipeline).
"""

import numpy as np
import ml_dtypes

import bass_rust as _br
import concourse.bass as bass
import concourse.mybir as mybir
import concourse.tile as tile
from concourse.bass_utils import run_bass_kernel_spmd
from concourse.vector_clock import ScopedClock

# ---------------------------------------------------------------------------
# Workaround: the walrus build in this container accepts at most ONE sync
# wait command per instruction ("Too many sync wait commands" in
# setupSyncWait).  Tile's scheduler freely attaches several waits per
# instruction.  Legalize at serialization time: rewrite the BIR JSON so any
# instruction with N>1 waits is preceded by N-1 single-wait NoOps on the
# same engine (waiting earlier on the same engine is always dependency-safe).
# ---------------------------------------------------------------------------
import json as _json

_orig_to_json_bytes = bass.Bass.to_json_bytes


def _lw_key(inst):
    return _json.dumps(
        {
            "ins": inst.get("ins"),
            "outs": inst.get("outs"),
            "perf_mode": inst.get("perf_mode"),
            "tile_position": inst.get("tile_position"),
            "tile_size": inst.get("tile_size"),
            "is_transpose": inst.get("is_transpose"),
        },
        sort_keys=True,
    )


def _dedupe_ldweights(bb):
    """Drop PE Ldweights whose weights are already loaded (identical to the
    previous Ldweights with only Matmult/seq ops in between).  The PE array
    retains its weights across matmuls, so the reload is pure sequencer
    drag (~71ns SW decode each); bass emits one per matmul unconditionally.
    A dropped Ldweights' waits/updates move onto the next PE instruction,
    which sits at the same position in the PE stream."""
    insts = bb.get("instructions", [])
    out = []
    last_lw = None
    pend_w, pend_u = [], []
    for inst in insts:
        if inst["engine"] != "PE":
            out.append(inst)
            continue
        op = inst["opcode"]
        if op == "Ldweights":
            key = _lw_key(inst)
            si = inst.get("sync_info") or {}
            if key == last_lw:
                pend_w.extend(si.get("on_wait") or [])
                pend_u.extend(si.get("on_update") or [])
                continue
            last_lw = key
            out.append(inst)
        else:
            if op not in ("Matmult", "NoOp"):
                last_lw = None  # be conservative around control flow etc.
            if pend_w or pend_u:
                si = inst.get("sync_info") or {}
                si["on_wait"] = pend_w + list(si.get("on_wait") or [])
                si["on_update"] = list(si.get("on_update") or []) + pend_u
                inst["sync_info"] = si
                pend_w, pend_u = [], []
            out.append(inst)
    if pend_w or pend_u:
        # no PE instruction followed; keep a NoOp to carry them
        out.append({
            "debug": 0, "engine": "PE", "ins": [], "outs": [],
            "name": "lwpend", "opcode": "NoOp",
            "sync_info": {"on_wait": pend_w, "on_update": pend_u},
        })
    bb["instructions"] = out


def _legalized_to_json_bytes(self):
    obj = _json.loads(_orig_to_json_bytes(self))
    for fn in obj.get("functions", []):
        for bb in fn.get("blocks", []):
            insts = bb.get("instructions", [])
            out = []
            changed = False
            for inst in insts:
                si = inst.get("sync_info")
                waits = (si or {}).get("on_wait") or []
                if len(waits) > 1:
                    changed = True
                    for k, w in enumerate(waits[:-1]):
                        out.append({
                            "debug": inst.get("debug", 0),
                            "engine": inst["engine"],
                            "ins": [],
                            "outs": [],
                            "name": f"{inst['name']}w{k}",
                            "opcode": "NoOp",
                            "sync_info": {"on_wait": [w], "on_update": []},
                        })
                    si["on_wait"] = [waits[-1]]
                out.append(inst)
            if changed:
                bb["instructions"] = out
    return _json.dumps(obj).encode()


bass.Bass.to_json_bytes = _legalized_to_json_bytes

# Also split the tail drain (it can carry many waits) so no single drain
# exceeds what the NoOp splitter above has to handle gracefully.
_MAX_DRAIN_WAITS = 4


def _split_drain_and_barrier(self, tick_clock, wait_clock):
    nc = self.nc
    drain_inst = nc.sync.drain()
    wait_clock.add_sem_waits(
        drain_inst.ins, ScopedClock({None: tick_clock.global_clock})
    )
    si = drain_inst.ins.sync_info
    if si is not None and len(si.on_wait) > _MAX_DRAIN_WAITS:
        waits = list(si.on_wait)
        ups = list(si.on_update)
        drain_inst.ins.sync_info = _br.SyncInfo(
            on_wait=waits[:_MAX_DRAIN_WAITS], on_update=[]
        )
        rest = waits[_MAX_DRAIN_WAITS:]
        while rest:
            chunk, rest = rest[:_MAX_DRAIN_WAITS], rest[_MAX_DRAIN_WAITS:]
            d2 = nc.sync.drain()
            d2.ins.sync_info = _br.SyncInfo(
                on_wait=chunk, on_update=([] if rest else ups)
            )
    nc.all_engine_barrier()
    assert self.sems is not None
    popped = nc._tile_sem_poison_stack.pop()
    assert popped is self._sem_poison
    nc.clear_and_free_semaphores(list(self.sems.allocated().values()))
    nc.all_engine_barrier()


tile.TileContext._drain_and_barrier = _split_drain_and_barrier

# (walrus's --enable-ldw-opt rejects bass's explicit InstLdweights, so the
# weight-load pipelining must come from scheduling: keep the PE instruction
# queue deep enough that each matmul's waits clear well before it issues.)

# ---------------------------------------------------------------------------
# Problem constants (hardcoded per the harness contract).
# ---------------------------------------------------------------------------
B, T, C = 4, 2048, 1024
NHEAD, HD = 16, 64          # total heads, head dim
NCORES = 8
TPG = 2                     # tensor-parallel groups (head groups)
HPC = NHEAD // TPG          # heads per core = 8
NQ = HPC * HD               # q (or k, or v) columns per core = 512
P = 128
SCALE = 1.0 / np.sqrt(HD)   # 0.125

BF16 = mybir.dt.bfloat16
F32 = mybir.dt.float32

_CACHE = {}


def _build_bass():
    nc = bass.Bass("TRN2")

    xT_d = nc.dram_tensor("xT", [C, T], BF16, kind="ExternalInput").ap()
    wqk_d = nc.dram_tensor("wqk", [C, 2 * NQ], BF16, kind="ExternalInput").ap()
    wv_d = nc.dram_tensor("wv", [C, NQ], BF16, kind="ExternalInput").ap()
    wp_d = nc.dram_tensor("wp", [NQ, C], BF16, kind="ExternalInput").ap()
    bqk_d = nc.dram_tensor("bqk", [2 * NQ, 1], F32, kind="ExternalInput").ap()
    bv_d = nc.dram_tensor("bv", [P, NQ], F32, kind="ExternalInput").ap()
    dmask_d = nc.dram_tensor("dmask", [P, P], BF16, kind="ExternalInput").ap()
    out_d = nc.dram_tensor("out", [T, C], BF16, kind="ExternalOutput").ap()

    CT = C // P        # 8 contraction tiles
    TT = T // P        # 16 t tiles
    TQ = 4             # T quarters (512 cols each)
    NQT = 2 * NQ // P  # 8 qk row tiles
    IW = 1024          # attention i-window width

    with tile.TileContext(nc) as tc:
        with tc.tile_pool(name="static", bufs=1) as st_pool:
            # ---- static SBUF residents ----
            xTq_sb = [[st_pool.tile([P, 512], BF16, name=f"xT{i}_{q}")
                       for q in range(TQ)] for i in range(CT)]
            wqk_sb = [st_pool.tile([P, 2 * NQ], BF16, name=f"wqk{i}") for i in range(CT)]
            wv_sb = [st_pool.tile([P, NQ], BF16, name=f"wv{i}") for i in range(CT)]
            wp_sb = [st_pool.tile([P, C], BF16, name=f"wp{i}") for i in range(NQ // P)]
            qkT_sb = [st_pool.tile([P, T], BF16, name=f"qkT{i}") for i in range(NQT)]
            vaug_sb = [st_pool.tile([P, 2 * NQ], BF16, name=f"vaug{i}") for i in range(TT)]
            yTh_sb = [[st_pool.tile([P, IW], BF16, name=f"yT{i}_{w}")
                       for w in range(T // IW)] for i in range(NQ // P)]
            bqk_sb = [st_pool.tile([P, 1], F32, name=f"bqk{i}") for i in range(NQT)]
            bv_sb = st_pool.tile([P, NQ], F32, name="bv")
            dmask_sb = st_pool.tile([P, P], BF16, name="dmask")

            # DMA order = need order.  HBM is the startup bound (~22us for
            # all inputs at ~330GB/s), so order strictly by first use:
            # the first GEMM chunks need wqk + xT quarter 0; v GEMMs need
            # wv + quarter 1; icb0 attention touches only t < 1024; the
            # later quarters, then wp (projection-only), stream in behind.
            for i in range(CT):
                nc.sync.dma_start(wqk_sb[i][:], wqk_d[P * i:P * (i + 1), :])
                nc.sync.dma_start(
                    xTq_sb[i][0][:], xT_d[P * i:P * (i + 1), 0:512]
                )
            for i in range(CT):
                nc.sync.dma_start(wv_sb[i][:], wv_d[P * i:P * (i + 1), :])
                nc.sync.dma_start(
                    xTq_sb[i][1][:], xT_d[P * i:P * (i + 1), 512:1024]
                )
            for i in range(NQT):
                nc.sync.dma_start(bqk_sb[i][:], bqk_d[P * i:P * (i + 1), :])
            nc.sync.dma_start(bv_sb[:], bv_d[:])
            nc.sync.dma_start(dmask_sb[:], dmask_d[:])
            for q in (2, 3):
                for i in range(CT):
                    nc.sync.dma_start(
                        xTq_sb[i][q][:],
                        xT_d[P * i:P * (i + 1), 512 * q:512 * (q + 1)],
                    )
            for i in range(NQ // P):
                nc.sync.dma_start(wp_sb[i][:], wp_d[P * i:P * (i + 1), :])
            for i in range(TT):
                vv = vaug_sb[i].rearrange("p (h x) -> p h x", x=2 * HD)
                nc.vector.memset(vv[:, :, HD:2 * HD], 1.0)

            def xT_cols(ct, t0, width):
                """xT[ct][:, t0:t0+width] as a view into the T-quartered
                tiles; (t0, width) must stay inside one 512-quarter."""
                q, o = divmod(t0, 512)
                assert o + width <= 512
                return xTq_sb[ct][q][:, o:o + width]

            # PSUM budget (8 banks of [128, 512] f32):
            #   poolST [128,1024] x2 bufs = 4 banks  (attention score tiles)
            #   poolAV [128,1024] x1 buf  = 2 banks  (attention accumulator)
            #   poolG  [128, 512] x2 bufs = 2 banks  (every GEMM chunk: qkv,
            #          v, proj -- double-buffered so the DVE bias-add of one
            #          chunk never stalls the matmuls of the next)
            with tc.tile_pool(name="poolST", bufs=2, space="PSUM") as poolST, \
                 tc.tile_pool(name="poolAV", bufs=1, space="PSUM") as poolAV, \
                 tc.tile_pool(name="poolG", bufs=2, space="PSUM") as poolG, \
                 tc.tile_pool(name="ptp", bufs=4) as ptp, \
                 tc.tile_pool(name="ysbp", bufs=3) as ysbp, \
                 tc.tile_pool(name="rbcp", bufs=4) as rbcp, \
                 tc.tile_pool(name="outp", bufs=4) as outp:

                def emit_qk_chunks(nt, cs):
                    # qkT[nt][:, 512c:512c+512] for c in cs = (x @ Wqk)^T+b.
                    # ct-major across the chunk pair so each wqk weight load
                    # serves len(cs) matmul streams.
                    pss = [poolG.tile([P, 512], F32, tag="g", name="ps_qk")
                           for _ in cs]
                    for ct in range(CT):
                        for ps, c in zip(pss, cs):
                            nc.tensor.matmul(
                                ps[:],
                                lhsT=wqk_sb[ct][:, P * nt:P * (nt + 1)],
                                rhs=xT_cols(ct, 512 * c, 512),
                                start=(ct == 0),
                                stop=(ct == CT - 1),
                            )
                    for ps, c in zip(pss, cs):
                        nc.vector.tensor_scalar_add(
                            qkT_sb[nt][:, 512 * c:512 * (c + 1)],
                            ps[:],
                            bqk_sb[nt][:, 0:1],
                        )

                def emit_proj_chunk(tt, mc):
                    ps = poolG.tile([P, 512], F32, tag="g", name="ps_p")
                    for n4 in range(NQ // P):
                        nc.tensor.matmul(
                            ps[:],
                            lhsT=yTh_sb[n4][tt // 8][:, P * (tt % 8):P * (tt % 8 + 1)],
                            rhs=wp_sb[n4][:, 512 * mc:512 * (mc + 1)],
                            start=(n4 == 0),
                            stop=(n4 == NQ // P - 1),
                        )
                    o_sb = outp.tile([P, 512], BF16, name="o")
                    if (2 * tt + mc) % 2 == 0:
                        nc.vector.tensor_copy(o_sb[:], ps[:])
                    else:
                        nc.scalar.copy(o_sb[:], ps[:])
                    nc.sync.dma_start(
                        out_d[P * tt:P * (tt + 1), 512 * mc:512 * (mc + 1)],
                        o_sb[:],
                    )

                def emit_v_chunk(tt):
                    ps = poolG.tile([P, 512], F32, tag="g", name="ps_v")
                    for ct in range(CT):
                        nc.tensor.matmul(
                            ps[:],
                            lhsT=xT_cols(ct, P * tt, P),
                            rhs=wv_sb[ct][:],
                            start=(ct == 0),
                            stop=(ct == CT - 1),
                        )
                    vv = vaug_sb[tt].rearrange("p (h x) -> p h x", x=2 * HD)
                    nc.vector.tensor_add(
                        vv[:, :, 0:HD],
                        ps[:].rearrange("p (h d) -> p h d", d=HD),
                        bv_sb.rearrange("p (h d) -> p h d", d=HD),
                    )

                # Deferred GEMM chunks, paced into the attention stream as PE
                # filler.  Order = need order; `fill_to` is also called with
                # hard deadlines before each consumer.  Pacing is adaptive:
                # spread the REMAINING chunks over the REMAINING slots, so a
                # deadline flush reduces the subsequent rate instead of
                # stalling it entirely (the v2 bug: slots//5 stayed below an
                # already-flushed `done` for dozens of slots).
                fillers = []
                for nt, c in ((0, 2), (0, 3), (4, 2), (4, 3)):
                    fillers.append(lambda nt=nt, c=c: emit_qk_chunks(nt, (c,)))
                for tt in range(8, TT):
                    fillers.append(lambda tt=tt: emit_v_chunk(tt))
                for nt in (1, 5, 2, 6, 3, 7):
                    for c in range(TQ):
                        fillers.append(lambda nt=nt, c=c: emit_qk_chunks(nt, (c,)))
                N_PREPROJ = len(fillers)
                for tt in range(8):
                    for mc in range(2):
                        fillers.append(
                            lambda tt=tt, mc=mc: emit_proj_chunk(tt, mc))
                TOTAL_SLOTS = 192
                fill = {"done": 0, "slots": 0, "last": 0, "cap": N_PREPROJ}

                def fill_to(target):
                    while fill["done"] < min(target, fill["cap"]):
                        fillers[fill["done"]]()
                        fill["done"] += 1
                        fill["last"] = fill["slots"]

                def slot_tick():
                    fill["slots"] += 1
                    left = fill["cap"] - fill["done"]
                    if left <= 0:
                        return
                    pace = max(1, (TOTAL_SLOTS - fill["slots"]) // (left + 1))
                    if fill["slots"] - fill["last"] >= pace:
                        fill_to(fill["done"] + 1)

                # Normalization is software-pipelined two windows deep so no
                # DVE op in the divide chain ever gates the PE stream.
                pending = []

                def norm_stage1(e):
                    # 1/den as exp(-ln(den)) on ScalarE: Ln and Exp share the
                    # natural_log_exp_and_others activation table (no reloads
                    # against the attention exps), each op is ~1.1us vs 6.5us
                    # for the exact DVE reciprocal (6 cycles/col), and ~5e-5
                    # relative accuracy is far inside the error budget.
                    t = rbcp.tile([HD, IW], F32, tag="rb", name="lnden")
                    rec_bc = rbcp.tile([HD, IW], F32, tag="rb", name="rec_bc")
                    nc.scalar.activation(
                        t[:], e["ysb"][HD:P, :],
                        mybir.ActivationFunctionType.Ln,
                    )
                    nc.scalar.activation(
                        rec_bc[:], t[:],
                        mybir.ActivationFunctionType.Exp,
                        scale=-1.0,
                    )
                    e["rec_bc"] = rec_bc

                def norm_stage2(e):
                    nc.vector.tensor_mul(
                        yTh_sb[e["qt"]][e["iwin"] // IW][e["qp"]:e["qp"] + HD, :],
                        e["ysb"][0:HD, :],
                        e["rec_bc"][:],
                    )

                def norm_flush():
                    if pending:
                        norm_stage1(pending[-1])
                    while pending:
                        norm_stage2(pending.pop(0))

                def emit_attention_window(h, icb):
                    qt, qp = h // 2, (h % 2) * HD
                    q_ap = qkT_sb[qt][qp:qp + HD, :]
                    k_ap = qkT_sb[4 + qt][qp:qp + HD, :]
                    iwin = IW * icb
                    jt_hi = (iwin + IW) // P  # exclusive
                    av = poolAV.tile([P, IW], F32, tag="av", name="av")
                    # last jt that touches each 512-wide bank of av
                    last_jt = [0, 0]
                    for jt in range(jt_hi):
                        off = max(0, P * jt - iwin)
                        for s in range(2):
                            if max(off, 512 * s) < 512 * (s + 1):
                                last_jt[s] = jt

                    def emit_av(jt, pt, off):
                        lhsT_av = vaug_sb[jt][:, 2 * HD * h:2 * HD * (h + 1)]
                        for s in range(2):
                            lo = max(off, 512 * s)
                            cw = 512 * (s + 1) - lo
                            if cw <= 0:
                                continue
                            nc.tensor.matmul(
                                av[:, lo:lo + cw],
                                lhsT=lhsT_av,
                                rhs=pt[:, lo:lo + cw],
                                start=(jt == 0),
                                stop=(jt == last_jt[s]),
                            )

                    # AV emission is skewed two j-tiles behind QK so each AV's
                    # exp (and diagonal mask) fires well before the PE reaches
                    # it -- a wait that clears just-in-time exposes the AV
                    # weight load serially (~107ns per tile).
                    pend_av = []
                    for jt in range(jt_hi):
                        off = max(0, P * jt - iwin)
                        st = poolST.tile([P, IW], F32, tag="w", name="st")
                        for s in range(2):
                            lo = max(off, 512 * s)
                            cw = 512 * (s + 1) - lo
                            if cw <= 0:
                                continue
                            nc.tensor.matmul(
                                st[:, lo:lo + cw],
                                lhsT=k_ap[:, P * jt:P * (jt + 1)],
                                rhs=q_ap[:, iwin + lo:iwin + lo + cw],
                                start=True,
                                stop=True,
                            )
                        pt = ptp.tile([P, IW], BF16, name="pt")
                        nc.scalar.activation(
                            pt[:, off:IW],
                            st[:, off:IW],
                            mybir.ActivationFunctionType.Exp,
                            scale=SCALE,
                        )
                        if P * jt >= iwin:
                            # diagonal block: zero the strictly-lower part
                            # (GpSimd: SBUF-only elementwise; keeps DVE free)
                            nc.gpsimd.tensor_mul(
                                pt[:, off:off + P],
                                pt[:, off:off + P],
                                dmask_sb[:],
                            )
                        pend_av.append((jt, pt, off))
                        if len(pend_av) > 1:
                            emit_av(*pend_av.pop(0))
                        slot_tick()
                    while pend_av:
                        emit_av(*pend_av.pop(0))
                    # one filler chunk here covers the av->ysb copy latency
                    fill_to(fill["done"] + 1)
                    # divide prep: yT = av[0:64] / av[64:128].  Copy av to
                    # SBUF (releases PSUM early); reciprocal_approx_fast on
                    # the replicated denominator rows; one DVE multiply.
                    ysb = ysbp.tile([P, IW], F32, tag="y", name="ysb")
                    nc.vector.tensor_copy(ysb[:], av[:])
                    pending.append(
                        {"ysb": ysb, "qt": qt, "qp": qp, "iwin": iwin}
                    )
                    if len(pending) >= 2:
                        norm_stage1(pending[-2])
                    if len(pending) >= 3:
                        norm_stage2(pending.pop(0))

                # ---- emission: the minimum prologue for h0.icb0 (q/k slabs
                # 0 and 4 over t<1024 plus v(0..7)), then attention windows
                # with the remaining GEMM chunks paced in.  icb0 windows only
                # touch t<1024, so the t>=1024 qk chunks defer to fillers.
                emit_qk_chunks(0, (0,))
                emit_qk_chunks(0, (1,))
                emit_qk_chunks(4, (0,))
                emit_qk_chunks(4, (1,))
                for tt in range(8):
                    emit_v_chunk(tt)
                for p in range(4):
                    if p >= 1:
                        fill_to(12 + 8 * p)  # qk(p), qk(p+4) must be done
                    for h in (2 * p, 2 * p + 1):
                        for icb in range(T // IW):
                            if (h, icb) == (0, 1):
                                fill_to(12)  # qk tails + v(8..15) first
                            if (h, icb) == (7, 1):
                                # drain the deferred normalizes so every
                                # icb0-half yT write is emitted BEFORE any
                                # projection filler reads it, then open the
                                # gate for the t<1024 projection chunks
                                norm_flush()
                                fill["cap"] = len(fillers)
                            emit_attention_window(h, icb)
                norm_flush()
                fill_to(len(fillers))

                # ---- phase E tail: the t>=1024 half of yT^T @ Wp (the
                # t<1024 half ran as fillers inside the last window) ----
                for tt in range(8, TT):
                    for mc in range(2):
                        emit_proj_chunk(tt, mc)

    return nc


def _prep_inputs(x, W_attn, b_attn, W_proj):
    """Per-core input maps (host-side shard + layout)."""
    bf16 = ml_dtypes.bfloat16
    dmask = np.triu(np.ones((P, P), np.float32)).astype(bf16)  # valid: col >= row
    in_maps = []
    for c in range(NCORES):
        b, g = c // TPG, c % TPG
        cols_q = slice(NQ * g, NQ * (g + 1))
        cols_k = slice(C + NQ * g, C + NQ * (g + 1))
        cols_v = slice(2 * C + NQ * g, 2 * C + NQ * (g + 1))
        xT = np.ascontiguousarray(x[b].T).astype(bf16)
        wqk = np.concatenate([W_attn[:, cols_q], W_attn[:, cols_k]], axis=1).astype(bf16)
        wv = np.ascontiguousarray(W_attn[:, cols_v]).astype(bf16)
        wp = np.ascontiguousarray(W_proj[NQ * g:NQ * (g + 1), :]).astype(bf16)
        bqk = np.concatenate([b_attn[cols_q], b_attn[cols_k]]).astype(np.float32)[:, None]
        bv = np.broadcast_to(b_attn[cols_v].astype(np.float32), (P, NQ)).copy()
        in_maps.append({
            "xT": xT, "wqk": wqk, "wv": wv, "wp": wp,
            "bqk": np.ascontiguousarray(bqk), "bv": bv, "dmask": dmask,
        })
    return in_maps


def _enable_tracing():
    """Install the NTFF profiling hook that the slim agent image lacks.

    Only needed for profiled runs (test harness); the plain kernel() path
    never calls this.  Replicates trn_boot's `_ntff_profile_via_ctypes`
    and stubs the (zero-egress) artifact upload.
    """
    import sys
    import types
    import ctypes
    import contextlib

    if "antenv.axon_hooks" not in sys.modules:
        import antenv

        mod = types.ModuleType("antenv.axon_hooks")
        box = {"h": None}
        mod.set_axon_ntff_profile_hook = lambda h: box.__setitem__("h", h)
        mod.get_axon_ntff_profile_hook = lambda: box["h"]
        sys.modules["antenv.axon_hooks"] = mod
        antenv.axon_hooks = mod

        so_path = "/opt/axon/libaxon_pjrt.so"
        lib = ctypes.CDLL(so_path)
        if hasattr(lib, "axon_start_nrt_profile"):
            lib.axon_start_nrt_profile.argtypes = [
                ctypes.POINTER(ctypes.c_int64),
                ctypes.c_size_t,
            ]
            lib.axon_start_nrt_profile.restype = ctypes.c_int64
            lib.axon_stop_nrt_profile.argtypes = [ctypes.c_char_p]
            lib.axon_stop_nrt_profile.restype = ctypes.c_int64

            @contextlib.contextmanager
            def _hook(output_dir, device_ids):
                import jax

                jax.devices()
                if device_ids:
                    ids = (ctypes.c_int64 * len(device_ids))(*device_ids)
                    rc = lib.axon_start_nrt_profile(ids, len(device_ids))
                else:
                    rc = lib.axon_start_nrt_profile(None, 0)
                if rc != 0:
                    raise RuntimeError(f"axon_start_nrt_profile rc={rc}")
                try:
                    yield
                finally:
                    n = lib.axon_stop_nrt_profile(str(output_dir).encode())
                    print(f"ntff profile: {n} file(s) -> {output_dir}")

            mod.set_axon_ntff_profile_hook(_hook)

    import concourse.bass_utils as bu

    bu.upload_artifacts = lambda tmpdir: tmpdir


def _run(in_maps, trace=False):
    if trace:
        _enable_tracing()
    if "nc" not in _CACHE:
        _CACHE["nc"] = _build_bass()
    return run_bass_kernel_spmd(
        _CACHE["nc"], in_maps, core_ids=list(range(NCORES)), trace=trace
    )


def kernel(x, W_attn, b_attn, W_proj, b_proj, _trace=False):
    x = np.asarray(x, dtype=np.float32)
    W_attn = np.asarray(W_attn, dtype=np.float32)
    b_attn = np.asarray(b_attn, dtype=np.float32)
    W_proj = np.asarray(W_proj, dtype=np.float32)
    b_proj = np.asarray(b_proj, dtype=np.float32)

    in_maps = _prep_inputs(x, W_attn, b_attn, W_proj)
    res = _run(in_maps, trace=_trace)
    out = np.empty((B, T, C), np.float32)
    for b in range(B):
        out[b] = (
            res.results[TPG * b]["out"].astype(np.float32)
            + res.results[TPG * b + 1]["out"].astype(np.float32)
            + b_proj
        )
    if _trace:
        kernel.last_exec_time_ns = res.exec_time_ns
        kernel.last_results = res
    return out


# revision 23
# speedup vs baseline: 1.0185x; 1.0185x over previous
"""Causal self-attention on 8 Trainium2 NeuronCores.

Problem: x[4, 2048, 1024] f32, W_attn[1024, 3072], b_attn[3072],
W_proj[1024, 1024], b_proj[1024];  16 heads, head_dim 64.

Sharding (data + tensor parallel, Megatron-style):
  core c = (b, g), b = c // 2 (batch), g = c % 2 (head group of 8 heads).
  - QKV weights column-sharded: core computes q,k,v for its 8 heads only.
  - W_proj row-sharded: core computes a partial [T, C] projection.
  - Host gathers: out[b] = partial[b,g=0] + partial[b,g=1] + b_proj.

Device layouts (per core):
  xTq  [128, 512] bf16 x (8 ct x 4 tq)  (x[b]^T quartered along T so the
                          first GEMM chunk only waits on 1/4 of the DMA)
  qkT  [1024, 2048] bf16 in SBUF: q rows 0-511, k rows 512-1023 (per-head
                          64-partition slabs -> ready as matmul operands)
  v    [2048, 1024] bf16: per head h a 128-col block [v_h (64) | ones (64)]
                          so the AV matmul lhsT yields PSUM rows 0-63 = y^T
                          and rows 64-127 = the softmax denominator
                          replicated 64x (free partition broadcast for the
                          divide).
  Causal: only blocks j <= i computed; diagonal 128x128 blocks masked by
  elementwise multiply with an upper-triangular 0/1 tile after exp.

Scheduling (v2): the ScalarE exp paces the attention inner loop (~0.85us
per j-tile vs ~0.6us of PE work), so PE micro-stalls every tile and each
stall drops the PE clock (p-state) for ~3us.  Fix: all GEMM PSUM chunks
are [128, 512] double-buffered (no DVE-add convoys), and the deferred
QKV GEMMs are paced INTO the attention stream one chunk per ~5 j-tiles
(with demand-driven flushes before each head-pair) so the PE queue always
holds runnable work.  The softmax normalization computes 1/den as
exp(-ln(den)) on ScalarE (Ln/Exp share one activation table, ~5e-5
accurate) -- the exact DVE reciprocal costs ~6 cycles/column and burned
105us of DVE, risking pacing the whole pipeline.  Output partials leave
the device in bf16 (the host sums the two head-group partials in f32).
"""

import numpy as np
import ml_dtypes

import bass_rust as _br
import concourse.bass as bass
import concourse.mybir as mybir
import concourse.tile as tile
from concourse.bass_utils import run_bass_kernel_spmd
from concourse.vector_clock import ScopedClock

# ---------------------------------------------------------------------------
# Workaround: the walrus build in this container accepts at most ONE sync
# wait command per instruction ("Too many sync wait commands" in
# setupSyncWait).  Tile's scheduler freely attaches several waits per
# instruction.  Legalize at serialization time: rewrite the BIR JSON so any
# instruction with N>1 waits is preceded by N-1 single-wait NoOps on the
# same engine (waiting earlier on the same engine is always dependency-safe).
# ---------------------------------------------------------------------------
import json as _json

_orig_to_json_bytes = bass.Bass.to_json_bytes


def _lw_key(inst):
    return _json.dumps(
        {
            "ins": inst.get("ins"),
            "outs": inst.get("outs"),
            "perf_mode": inst.get("perf_mode"),
            "tile_position": inst.get("tile_position"),
            "tile_size": inst.get("tile_size"),
            "is_transpose": inst.get("is_transpose"),
        },
        sort_keys=True,
    )


def _dedupe_ldweights(bb):
    """Drop PE Ldweights whose weights are already loaded (identical to the
    previous Ldweights with only Matmult/seq ops in between).  The PE array
    retains its weights across matmuls, so the reload is pure sequencer
    drag (~71ns SW decode each); bass emits one per matmul unconditionally.
    A dropped Ldweights' waits/updates move onto the next PE instruction,
    which sits at the same position in the PE stream."""
    insts = bb.get("instructions", [])
    out = []
    last_lw = None
    pend_w, pend_u = [], []
    for inst in insts:
        if inst["engine"] != "PE":
            out.append(inst)
            continue
        op = inst["opcode"]
        if op == "Ldweights":
            key = _lw_key(inst)
            si = inst.get("sync_info") or {}
            if key == last_lw:
                pend_w.extend(si.get("on_wait") or [])
                pend_u.extend(si.get("on_update") or [])
                continue
            last_lw = key
            out.append(inst)
        else:
            if op not in ("Matmult", "NoOp"):
                last_lw = None  # be conservative around control flow etc.
            if pend_w or pend_u:
                si = inst.get("sync_info") or {}
                si["on_wait"] = pend_w + list(si.get("on_wait") or [])
                si["on_update"] = list(si.get("on_update") or []) + pend_u
                inst["sync_info"] = si
                pend_w, pend_u = [], []
            out.append(inst)
    if pend_w or pend_u:
        # no PE instruction followed; keep a NoOp to carry them
        out.append({
            "debug": 0, "engine": "PE", "ins": [], "outs": [],
            "name": "lwpend", "opcode": "NoOp",
            "sync_info": {"on_wait": pend_w, "on_update": pend_u},
        })
    bb["instructions"] = out


def _legalized_to_json_bytes(self):
    obj = _json.loads(_orig_to_json_bytes(self))
    for fn in obj.get("functions", []):
        for bb in fn.get("blocks", []):
            insts = bb.get("instructions", [])
            out = []
            changed = False
            for inst in insts:
                si = inst.get("sync_info")
                waits = (si or {}).get("on_wait") or []
                if len(waits) > 1:
                    changed = True
                    for k, w in enumerate(waits[:-1]):
                        out.append({
                            "debug": inst.get("debug", 0),
                            "engine": inst["engine"],
                            "ins": [],
                            "outs": [],
                            "name": f"{inst['name']}w{k}",
                            "opcode": "NoOp",
                            "sync_info": {"on_wait": [w], "on_update": []},
                        })
                    si["on_wait"] = [waits[-1]]
                out.append(inst)
            if changed:
                bb["instructions"] = out
    return _json.dumps(obj).encode()


bass.Bass.to_json_bytes = _legalized_to_json_bytes

# Also split the tail drain (it can carry many waits) so no single drain
# exceeds what the NoOp splitter above has to handle gracefully.
_MAX_DRAIN_WAITS = 4


def _split_drain_and_barrier(self, tick_clock, wait_clock):
    nc = self.nc
    drain_inst = nc.sync.drain()
    wait_clock.add_sem_waits(
        drain_inst.ins, ScopedClock({None: tick_clock.global_clock})
    )
    si = drain_inst.ins.sync_info
    if si is not None and len(si.on_wait) > _MAX_DRAIN_WAITS:
        waits = list(si.on_wait)
        ups = list(si.on_update)
        drain_inst.ins.sync_info = _br.SyncInfo(
            on_wait=waits[:_MAX_DRAIN_WAITS], on_update=[]
        )
        rest = waits[_MAX_DRAIN_WAITS:]
        while rest:
            chunk, rest = rest[:_MAX_DRAIN_WAITS], rest[_MAX_DRAIN_WAITS:]
            d2 = nc.sync.drain()
            d2.ins.sync_info = _br.SyncInfo(
                on_wait=chunk, on_update=([] if rest else ups)
            )
    nc.all_engine_barrier()
    assert self.sems is not None
    popped = nc._tile_sem_poison_stack.pop()
    assert popped is self._sem_poison
    nc.clear_and_free_semaphores(list(self.sems.allocated().values()))
    nc.all_engine_barrier()


tile.TileContext._drain_and_barrier = _split_drain_and_barrier

# (walrus's --enable-ldw-opt rejects bass's explicit InstLdweights, so the
# weight-load pipelining must come from scheduling: keep the PE instruction
# queue deep enough that each matmul's waits clear well before it issues.)

# ---------------------------------------------------------------------------
# Problem constants (hardcoded per the harness contract).
# ---------------------------------------------------------------------------
B, T, C = 4, 2048, 1024
NHEAD, HD = 16, 64          # total heads, head dim
NCORES = 8
TPG = 2                     # tensor-parallel groups (head groups)
HPC = NHEAD // TPG          # heads per core = 8
NQ = HPC * HD               # q (or k, or v) columns per core = 512
P = 128
SCALE = 1.0 / np.sqrt(HD)   # 0.125

BF16 = mybir.dt.bfloat16
F32 = mybir.dt.float32

_CACHE = {}


def _build_bass():
    nc = bass.Bass("TRN2")

    xT_d = nc.dram_tensor("xT", [C, T], BF16, kind="ExternalInput").ap()
    wqk_d = nc.dram_tensor("wqk", [C, 2 * NQ], BF16, kind="ExternalInput").ap()
    wv_d = nc.dram_tensor("wv", [C, NQ], BF16, kind="ExternalInput").ap()
    wp_d = nc.dram_tensor("wp", [NQ, C], BF16, kind="ExternalInput").ap()
    bqk_d = nc.dram_tensor("bqk", [2 * NQ, 1], F32, kind="ExternalInput").ap()
    bv_d = nc.dram_tensor("bv", [P, NQ], F32, kind="ExternalInput").ap()
    dmask_d = nc.dram_tensor("dmask", [P, P], BF16, kind="ExternalInput").ap()
    out_d = nc.dram_tensor("out", [T, C], BF16, kind="ExternalOutput").ap()

    CT = C // P        # 8 contraction tiles
    TT = T // P        # 16 t tiles
    TQ = 4             # T quarters (512 cols each)
    NQT = 2 * NQ // P  # 8 qk row tiles
    IW = 1024          # attention i-window width

    with tile.TileContext(nc) as tc:
        with tc.tile_pool(name="static", bufs=1) as st_pool:
            # ---- static SBUF residents ----
            xTq_sb = [[st_pool.tile([P, 512], BF16, name=f"xT{i}_{q}")
                       for q in range(TQ)] for i in range(CT)]
            wqk_sb = [st_pool.tile([P, 2 * NQ], BF16, name=f"wqk{i}") for i in range(CT)]
            wv_sb = [st_pool.tile([P, NQ], BF16, name=f"wv{i}") for i in range(CT)]
            wp_sb = [st_pool.tile([P, C], BF16, name=f"wp{i}") for i in range(NQ // P)]
            qkT_sb = [st_pool.tile([P, T], BF16, name=f"qkT{i}") for i in range(NQT)]
            vaug_sb = [st_pool.tile([P, 2 * NQ], BF16, name=f"vaug{i}") for i in range(TT)]
            yT_sb = [st_pool.tile([P, T], BF16, name=f"yT{i}") for i in range(NQ // P)]
            bqk_sb = [st_pool.tile([P, 1], F32, name=f"bqk{i}") for i in range(NQT)]
            bv_sb = st_pool.tile([P, NQ], F32, name="bv")
            dmask_sb = st_pool.tile([P, P], BF16, name="dmask")

            # DMA order = need order.  HBM is the startup bound (~22us for
            # all inputs at ~330GB/s), so order strictly by first use:
            # the first GEMM chunks need wqk + xT quarter 0; v GEMMs need
            # wv + quarter 1; icb0 attention touches only t < 1024; the
            # later quarters, then wp (projection-only), stream in behind.
            for i in range(CT):
                nc.sync.dma_start(wqk_sb[i][:], wqk_d[P * i:P * (i + 1), :])
                nc.sync.dma_start(
                    xTq_sb[i][0][:], xT_d[P * i:P * (i + 1), 0:512]
                )
            for i in range(CT):
                nc.sync.dma_start(wv_sb[i][:], wv_d[P * i:P * (i + 1), :])
                nc.sync.dma_start(
                    xTq_sb[i][1][:], xT_d[P * i:P * (i + 1), 512:1024]
                )
            for i in range(NQT):
                nc.sync.dma_start(bqk_sb[i][:], bqk_d[P * i:P * (i + 1), :])
            nc.sync.dma_start(bv_sb[:], bv_d[:])
            nc.sync.dma_start(dmask_sb[:], dmask_d[:])
            for q in (2, 3):
                for i in range(CT):
                    nc.sync.dma_start(
                        xTq_sb[i][q][:],
                        xT_d[P * i:P * (i + 1), 512 * q:512 * (q + 1)],
                    )
            for i in range(NQ // P):
                nc.sync.dma_start(wp_sb[i][:], wp_d[P * i:P * (i + 1), :])
            for i in range(TT):
                vv = vaug_sb[i].rearrange("p (h x) -> p h x", x=2 * HD)
                nc.vector.memset(vv[:, :, HD:2 * HD], 1.0)

            def xT_cols(ct, t0, width):
                """xT[ct][:, t0:t0+width] as a view into the T-quartered
                tiles; (t0, width) must stay inside one 512-quarter."""
                q, o = divmod(t0, 512)
                assert o + width <= 512
                return xTq_sb[ct][q][:, o:o + width]

            # PSUM budget (8 banks of [128, 512] f32):
            #   poolST [128,1024] x2 bufs = 4 banks  (attention score tiles)
            #   poolAV [128,1024] x1 buf  = 2 banks  (attention accumulator)
            #   poolG  [128, 512] x2 bufs = 2 banks  (every GEMM chunk: qkv,
            #          v, proj -- double-buffered so the DVE bias-add of one
            #          chunk never stalls the matmuls of the next)
            with tc.tile_pool(name="poolST", bufs=2, space="PSUM") as poolST, \
                 tc.tile_pool(name="poolAV", bufs=1, space="PSUM") as poolAV, \
                 tc.tile_pool(name="poolG", bufs=2, space="PSUM") as poolG, \
                 tc.tile_pool(name="ptp", bufs=4) as ptp, \
                 tc.tile_pool(name="ysbp", bufs=3) as ysbp, \
                 tc.tile_pool(name="rbcp", bufs=4) as rbcp, \
                 tc.tile_pool(name="outp", bufs=4) as outp:

                def emit_qk_chunks(nt, cs):
                    # qkT[nt][:, 512c:512c+512] for c in cs = (x @ Wqk)^T+b.
                    # ct-major across the chunk pair so each wqk weight load
                    # serves len(cs) matmul streams.
                    pss = [poolG.tile([P, 512], F32, tag="g", name="ps_qk")
                           for _ in cs]
                    for ct in range(CT):
                        for ps, c in zip(pss, cs):
                            nc.tensor.matmul(
                                ps[:],
                                lhsT=wqk_sb[ct][:, P * nt:P * (nt + 1)],
                                rhs=xT_cols(ct, 512 * c, 512),
                                start=(ct == 0),
                                stop=(ct == CT - 1),
                            )
                    for ps, c in zip(pss, cs):
                        nc.vector.tensor_scalar_add(
                            qkT_sb[nt][:, 512 * c:512 * (c + 1)],
                            ps[:],
                            bqk_sb[nt][:, 0:1],
                        )

                def emit_v_chunk(tt):
                    ps = poolG.tile([P, 512], F32, tag="g", name="ps_v")
                    for ct in range(CT):
                        nc.tensor.matmul(
                            ps[:],
                            lhsT=xT_cols(ct, P * tt, P),
                            rhs=wv_sb[ct][:],
                            start=(ct == 0),
                            stop=(ct == CT - 1),
                        )
                    vv = vaug_sb[tt].rearrange("p (h x) -> p h x", x=2 * HD)
                    nc.vector.tensor_add(
                        vv[:, :, 0:HD],
                        ps[:].rearrange("p (h d) -> p h d", d=HD),
                        bv_sb.rearrange("p (h d) -> p h d", d=HD),
                    )

                # Deferred GEMM chunks, paced into the attention stream as PE
                # filler.  Order = need order; `fill_to` is also called with
                # hard deadlines before each consumer.  Pacing is adaptive:
                # spread the REMAINING chunks over the REMAINING slots, so a
                # deadline flush reduces the subsequent rate instead of
                # stalling it entirely (the v2 bug: slots//5 stayed below an
                # already-flushed `done` for dozens of slots).
                fillers = []
                for nt, c in ((0, 2), (0, 3), (4, 2), (4, 3)):
                    fillers.append(lambda nt=nt, c=c: emit_qk_chunks(nt, (c,)))
                for tt in range(8, TT):
                    fillers.append(lambda tt=tt: emit_v_chunk(tt))
                for nt in (1, 5, 2, 6, 3, 7):
                    for c in range(TQ):
                        fillers.append(lambda nt=nt, c=c: emit_qk_chunks(nt, (c,)))
                TOTAL_SLOTS = 192
                fill = {"done": 0, "slots": 0, "last": 0}

                def fill_to(target):
                    while fill["done"] < min(target, len(fillers)):
                        fillers[fill["done"]]()
                        fill["done"] += 1
                        fill["last"] = fill["slots"]

                def slot_tick():
                    fill["slots"] += 1
                    left = len(fillers) - fill["done"]
                    if left <= 0:
                        return
                    pace = max(1, (TOTAL_SLOTS - fill["slots"]) // (left + 1))
                    if fill["slots"] - fill["last"] >= pace:
                        fill_to(fill["done"] + 1)

                # Normalization is software-pipelined two windows deep so no
                # DVE op in the divide chain ever gates the PE stream.
                pending = []

                def norm_stage1(e):
                    # 1/den as exp(-ln(den)) on ScalarE: Ln and Exp share the
                    # natural_log_exp_and_others activation table (no reloads
                    # against the attention exps), each op is ~1.1us vs 6.5us
                    # for the exact DVE reciprocal (6 cycles/col), and ~5e-5
                    # relative accuracy is far inside the error budget.
                    t = rbcp.tile([HD, IW], F32, tag="rb", name="lnden")
                    rec_bc = rbcp.tile([HD, IW], F32, tag="rb", name="rec_bc")
                    nc.scalar.activation(
                        t[:], e["ysb"][HD:P, :],
                        mybir.ActivationFunctionType.Ln,
                    )
                    nc.scalar.activation(
                        rec_bc[:], t[:],
                        mybir.ActivationFunctionType.Exp,
                        scale=-1.0,
                    )
                    e["rec_bc"] = rec_bc

                def norm_stage2(e):
                    nc.vector.tensor_mul(
                        yT_sb[e["qt"]][e["qp"]:e["qp"] + HD,
                                       e["iwin"]:e["iwin"] + IW],
                        e["ysb"][0:HD, :],
                        e["rec_bc"][:],
                    )

                def norm_flush():
                    if pending:
                        norm_stage1(pending[-1])
                    while pending:
                        norm_stage2(pending.pop(0))

                def emit_attention_window(h, icb):
                    qt, qp = h // 2, (h % 2) * HD
                    q_ap = qkT_sb[qt][qp:qp + HD, :]
                    k_ap = qkT_sb[4 + qt][qp:qp + HD, :]
                    iwin = IW * icb
                    jt_hi = (iwin + IW) // P  # exclusive
                    av = poolAV.tile([P, IW], F32, tag="av", name="av")
                    # last jt that touches each 512-wide bank of av
                    last_jt = [0, 0]
                    for jt in range(jt_hi):
                        off = max(0, P * jt - iwin)
                        for s in range(2):
                            if max(off, 512 * s) < 512 * (s + 1):
                                last_jt[s] = jt

                    def emit_av(jt, pt, off):
                        lhsT_av = vaug_sb[jt][:, 2 * HD * h:2 * HD * (h + 1)]
                        for s in range(2):
                            lo = max(off, 512 * s)
                            cw = 512 * (s + 1) - lo
                            if cw <= 0:
                                continue
                            nc.tensor.matmul(
                                av[:, lo:lo + cw],
                                lhsT=lhsT_av,
                                rhs=pt[:, lo:lo + cw],
                                start=(jt == 0),
                                stop=(jt == last_jt[s]),
                            )

                    # AV emission is skewed two j-tiles behind QK so each AV's
                    # exp (and diagonal mask) fires well before the PE reaches
                    # it -- a wait that clears just-in-time exposes the AV
                    # weight load serially (~107ns per tile).
                    pend_av = []
                    for jt in range(jt_hi):
                        off = max(0, P * jt - iwin)
                        st = poolST.tile([P, IW], F32, tag="w", name="st")
                        for s in range(2):
                            lo = max(off, 512 * s)
                            cw = 512 * (s + 1) - lo
                            if cw <= 0:
                                continue
                            nc.tensor.matmul(
                                st[:, lo:lo + cw],
                                lhsT=k_ap[:, P * jt:P * (jt + 1)],
                                rhs=q_ap[:, iwin + lo:iwin + lo + cw],
                                start=True,
                                stop=True,
                            )
                        pt = ptp.tile([P, IW], BF16, name="pt")
                        nc.scalar.activation(
                            pt[:, off:IW],
                            st[:, off:IW],
                            mybir.ActivationFunctionType.Exp,
                            scale=SCALE,
                        )
                        if P * jt >= iwin:
                            # diagonal block: zero the strictly-lower part
                            # (GpSimd: SBUF-only elementwise; keeps DVE free)
                            nc.gpsimd.tensor_mul(
                                pt[:, off:off + P],
                                pt[:, off:off + P],
                                dmask_sb[:],
                            )
                        pend_av.append((jt, pt, off))
                        if len(pend_av) > 1:
                            emit_av(*pend_av.pop(0))
                        slot_tick()
                    while pend_av:
                        emit_av(*pend_av.pop(0))
                    # one filler chunk here covers the av->ysb copy latency
                    fill_to(fill["done"] + 1)
                    # divide prep: yT = av[0:64] / av[64:128].  Copy av to
                    # SBUF (releases PSUM early); reciprocal_approx_fast on
                    # the replicated denominator rows; one DVE multiply.
                    ysb = ysbp.tile([P, IW], F32, tag="y", name="ysb")
                    nc.vector.tensor_copy(ysb[:], av[:])
                    pending.append(
                        {"ysb": ysb, "qt": qt, "qp": qp, "iwin": iwin}
                    )
                    if len(pending) >= 2:
                        norm_stage1(pending[-2])
                    if len(pending) >= 3:
                        norm_stage2(pending.pop(0))

                # ---- emission: the minimum prologue for h0.icb0 (q/k slabs
                # 0 and 4 over t<1024 plus v(0..7)), then attention windows
                # with the remaining GEMM chunks paced in.  icb0 windows only
                # touch t<1024, so the t>=1024 qk chunks defer to fillers.
                emit_qk_chunks(0, (0,))
                emit_qk_chunks(0, (1,))
                emit_qk_chunks(4, (0,))
                emit_qk_chunks(4, (1,))
                for tt in range(8):
                    emit_v_chunk(tt)
                for p in range(4):
                    if p >= 1:
                        fill_to(12 + 8 * p)  # qk(p), qk(p+4) must be done
                    for h in (2 * p, 2 * p + 1):
                        for icb in range(T // IW):
                            if (h, icb) == (0, 1):
                                fill_to(12)  # qk tails + v(8..15) first
                            emit_attention_window(h, icb)
                norm_flush()

                # ---- phase E: partial = yT^T @ Wp ----
                for tt in range(TT):
                    for mc in range(2):
                        ps = poolG.tile([P, 512], F32, tag="g", name="ps_p")
                        for n4 in range(NQ // P):
                            nc.tensor.matmul(
                                ps[:],
                                lhsT=yT_sb[n4][:, P * tt:P * (tt + 1)],
                                rhs=wp_sb[n4][:, 512 * mc:512 * (mc + 1)],
                                start=(n4 == 0),
                                stop=(n4 == NQ // P - 1),
                            )
                        o_sb = outp.tile([P, 512], BF16, name="o")
                        if (2 * tt + mc) % 2 == 0:
                            nc.vector.tensor_copy(o_sb[:], ps[:])
                        else:
                            nc.scalar.copy(o_sb[:], ps[:])
                        nc.sync.dma_start(
                            out_d[P * tt:P * (tt + 1), 512 * mc:512 * (mc + 1)],
                            o_sb[:],
                        )

    return nc


def _prep_inputs(x, W_attn, b_attn, W_proj):
    """Per-core input maps (host-side shard + layout)."""
    bf16 = ml_dtypes.bfloat16
    dmask = np.triu(np.ones((P, P), np.float32)).astype(bf16)  # valid: col >= row
    in_maps = []
    for c in range(NCORES):
        b, g = c // TPG, c % TPG
        cols_q = slice(NQ * g, NQ * (g + 1))
        cols_k = slice(C + NQ * g, C + NQ * (g + 1))
        cols_v = slice(2 * C + NQ * g, 2 * C + NQ * (g + 1))
        xT = np.ascontiguousarray(x[b].T).astype(bf16)
        wqk = np.concatenate([W_attn[:, cols_q], W_attn[:, cols_k]], axis=1).astype(bf16)
        wv = np.ascontiguousarray(W_attn[:, cols_v]).astype(bf16)
        wp = np.ascontiguousarray(W_proj[NQ * g:NQ * (g + 1), :]).astype(bf16)
        bqk = np.concatenate([b_attn[cols_q], b_attn[cols_k]]).astype(np.float32)[:, None]
        bv = np.broadcast_to(b_attn[cols_v].astype(np.float32), (P, NQ)).copy()
        in_maps.append({
            "xT": xT, "wqk": wqk, "wv": wv, "wp": wp,
            "bqk": np.ascontiguousarray(bqk), "bv": bv, "dmask": dmask,
        })
    return in_maps


def _enable_tracing():
    """Install the NTFF profiling hook that the slim agent image lacks.

    Only needed for profiled runs (test harness); the plain kernel() path
    never calls this.  Replicates trn_boot's `_ntff_profile_via_ctypes`
    and stubs the (zero-egress) artifact upload.
    """
    import sys
    import types
    import ctypes
    import contextlib

    if "antenv.axon_hooks" not in sys.modules:
        import antenv

        mod = types.ModuleType("antenv.axon_hooks")
        box = {"h": None}
        mod.set_axon_ntff_profile_hook = lambda h: box.__setitem__("h", h)
        mod.get_axon_ntff_profile_hook = lambda: box["h"]
        sys.modules["antenv.axon_hooks"] = mod
        antenv.axon_hooks = mod

        so_path = "/opt/axon/libaxon_pjrt.so"
        lib = ctypes.CDLL(so_path)
        if hasattr(lib, "axon_start_nrt_profile"):
            lib.axon_start_nrt_profile.argtypes = [
                ctypes.POINTER(ctypes.c_int64),
                ctypes.c_size_t,
            ]
            lib.axon_start_nrt_profile.restype = ctypes.c_int64
            lib.axon_stop_nrt_profile.argtypes = [ctypes.c_char_p]
            lib.axon_stop_nrt_profile.restype = ctypes.c_int64

            @contextlib.contextmanager
            def _hook(output_dir, device_ids):
                import jax

                jax.devices()
                if device_ids:
                    ids = (ctypes.c_int64 * len(device_ids))(*device_ids)
                    rc = lib.axon_start_nrt_profile(ids, len(device_ids))
                else:
                    rc = lib.axon_start_nrt_profile(None, 0)
                if rc != 0:
                    raise RuntimeError(f"axon_start_nrt_profile rc={rc}")
                try:
                    yield
                finally:
                    n = lib.axon_stop_nrt_profile(str(output_dir).encode())
                    print(f"ntff profile: {n} file(s) -> {output_dir}")

            mod.set_axon_ntff_profile_hook(_hook)

    import concourse.bass_utils as bu

    bu.upload_artifacts = lambda tmpdir: tmpdir


def _run(in_maps, trace=False):
    if trace:
        _enable_tracing()
    if "nc" not in _CACHE:
        _CACHE["nc"] = _build_bass()
    return run_bass_kernel_spmd(
        _CACHE["nc"], in_maps, core_ids=list(range(NCORES)), trace=trace
    )


def kernel(x, W_attn, b_attn, W_proj, b_proj, _trace=False):
    x = np.asarray(x, dtype=np.float32)
    W_attn = np.asarray(W_attn, dtype=np.float32)
    b_attn = np.asarray(b_attn, dtype=np.float32)
    W_proj = np.asarray(W_proj, dtype=np.float32)
    b_proj = np.asarray(b_proj, dtype=np.float32)

    in_maps = _prep_inputs(x, W_attn, b_attn, W_proj)
    res = _run(in_maps, trace=_trace)
    out = np.empty((B, T, C), np.float32)
    for b in range(B):
        out[b] = (
            res.results[TPG * b]["out"].astype(np.float32)
            + res.results[TPG * b + 1]["out"].astype(np.float32)
            + b_proj
        )
    if _trace:
        kernel.last_exec_time_ns = res.exec_time_ns
        kernel.last_results = res
    return out
